# revision 22
# baseline (speedup 1.0000x reference)
"""Linformer self-attention on 8 Trainium2 NeuronCores.

Sharding: core = (batch b, head-group g) with b = core//2, g = core%2.
Each core computes attention for batch b and its 8 heads (512 of the 1024
channels), then a row-sharded W_out matmul producing a partial output in
transposed [slab, 128, jc, n] fp16 layout; the host sums the two partials
per batch in f32, transposes once, and adds b_out.

Linformer associativity: k_proj = E^T (x Wk) = (E^T x) Wk (same for v), so
xET = (E^T x)^T is computed once and shared by the k and v projections.

v2 restructure vs baseline (which was Scalar/Vector-bound at ~4.6us/step):
  - Heads loop processes HEAD PAIRS (2c, 2c+1): their score matmuls use
    contraction rows 0-63 / 64-127 (auto row-tiling -> concurrent), their
    AV matmuls write psum partitions 0-63 / 64-127 (auto col-tiling ->
    concurrent), halving PE time for both stages.
  - exp on ScalarE emits the softmax denominator via accum_out (no
    reduce_sum on Vector).
  - Normalization is folded into the transpose: U^T @ diag(1/denom) as a
    regular matmul (rhs = diag built on the otherwise-idle GpSimd from
    id16 * srecip), so no separate normalize pass exists.
  - PSUM evacuations balanced across Scalar and Vector; av evacuated once
    per pair at full width.
  - DMAs batched (4 n-tiles per xn load, one load per xT slab, one store
    per output slab) to unclog the Sync queue that paced the front epoch.

Precision: q/k score chain f32r; U/vp/attn-out/W_out/output fp16.
"""

import os
import numpy as np

import concourse.bacc as bacc
import concourse.tile as tile
from concourse import mybir
from concourse.bass_utils import run_bass_kernel_spmd

F32 = mybir.dt.float32
F16 = mybir.dt.float16
F32R = mybir.dt.float32r
EXP = mybir.ActivationFunctionType.Exp
AXX = mybir.AxisListType.X

DIM, SEQ, KR, HD = 1024, 4096, 256, 64
CG = 512               # channels per head-group (8 heads x 64)
NSLAB = 512
SLABS = SEQ // NSLAB   # 8
NT = SEQ // 128        # 32 natural n-tiles
NG = NT // 4           # 8 groups of 4 n-tiles (one DMA each)
DC = DIM // 128        # 8 d-tiles
SCALE = HD ** -0.5
NPAIR = SLABS * 4      # 32 head-pair steps

_cache = {}


def build_program():
    nc = bacc.Bacc("TRN2", target_bir_lowering=False, debug=False, num_devices=8)

    xn_d = nc.dram_tensor("xn", [NG, 128, 4, DIM], F16, kind="ExternalInput")
    Ed = nc.dram_tensor("E", [NG, 128, 4, KR], F16, kind="ExternalInput")
    xT_d = nc.dram_tensor("xT", [SLABS, 128, DC, NSLAB], F16, kind="ExternalInput")
    Wq = nc.dram_tensor("Wq", [128, DC, CG], F16, kind="ExternalInput")
    Wk = nc.dram_tensor("Wk", [128, DC, CG], F32R, kind="ExternalInput")
    Wv = nc.dram_tensor("Wv", [128, DC, CG], F32R, kind="ExternalInput")
    Wo = nc.dram_tensor("Wo", [128, 4, DIM], F16, kind="ExternalInput")
    id32_d = nc.dram_tensor("id32", [128, 128], F32R, kind="ExternalInput")
    id16_d = nc.dram_tensor("id16", [128, 128], F16, kind="ExternalInput")
    out_d = nc.dram_tensor("out", [SLABS, 128, 8, NSLAB], F16,
                           kind="ExternalOutput")
    dbg = os.environ.get("KERNEL_DEBUG", "0") == "1"
    if dbg:
        dbg_xET = nc.dram_tensor("dbg_xET", [128, DC, KR], F32R,
                                 kind="ExternalOutput")
        dbg_kpT = nc.dram_tensor("dbg_kpT", [128, 4, KR], F16,
                                 kind="ExternalOutput")
        dbg_vp = nc.dram_tensor("dbg_vp", [128, 2, CG], F16,
                                kind="ExternalOutput")
        dbg_qt = nc.dram_tensor("dbg_qt", [128, 4, 2, NSLAB], F16,
                                kind="ExternalOutput")

    mm = nc.tensor.matmul

    with tile.TileContext(nc) as tc:
        with tc.tile_pool(name="const", bufs=1) as const:
            wq_sb = const.tile([128, DC, CG], F16)
            wo_sb = const.tile([128, 4, DIM], F16)
            id32_sb = const.tile([128, 128], F32R)
            id16_sb = const.tile([128, 128], F16)
            xET_sb = const.tile([128, DC, KR], F32R)   # x^T E  [d, kr]
            kpT_sb = const.tile([128, 4, KR], F16)     # (kp)^T [c, kr]
            vp_sb = const.tile([128, 2, CG], F16)      # vp     [kr, c]
            qt = const.tile([128, 4, SLABS, NSLAB], F16)   # q^T, all slabs

            nc.sync.dma_start(out=id32_sb, in_=id32_d[:, :])
            nc.sync.dma_start(out=id16_sb, in_=id16_d[:, :])
            nc.sync.dma_start(out=wq_sb, in_=Wq[:, :, :])

            # ---------------- FRONT: xET accumulate + kpT/vp + qT(s=0,1) ----
            with tc.tile_pool(name="frA", bufs=1) as frA, \
                 tc.tile_pool(name="psQ", bufs=1, space="PSUM") as psQ:
                wk_sb = frA.tile([128, DC, CG], F32R)
                wv_sb = frA.tile([128, DC, CG], F32R)

                def rr_copy(i, out, in_):
                    eng = (nc.vector.tensor_copy, nc.scalar.copy)[i % 2]
                    eng(out, in_)

                # xs for slabs 0/1 early: qT chunks fill xE's DMA bubbles
                xs_f = []
                for s in range(2):
                    xs = const.tile([128, DC, NSLAB], F16, name=f"xs_{s}")
                    nc.sync.dma_start(out=xs, in_=xT_d[s])
                    xs_f.append(xs)

                def qT_front(s, ct):
                    q_ps = psQ.tile([128, NSLAB], F32, tag="qps",
                                    bufs=2, name=f"qps_{s}_{ct}")
                    for dc in range(DC):
                        mm(q_ps,
                           lhsT=wq_sb[:, dc, ct * 128:(ct + 1) * 128],
                           rhs=xs_f[s][:, dc, :],
                           start=(dc == 0), stop=(dc == DC - 1))
                    rr_copy(ct, qt[:, ct, s, :], q_ps)

                # qT chunk schedule for slab 0 between xE groups (slab 1
                # is streamed at the start of the heads epoch)
                q_front_sched = {3: (0, 0), 4: (0, 1), 5: (0, 2), 6: (0, 3)}

                with tc.tile_pool(name="psE", bufs=1, space="PSUM") as psE:
                    # HAM warmup: keep the PE busy while x streams in so the
                    # clock gate opens before the real matmuls arrive
                    for w in range(6):
                        wt = psQ.tile([128, 4, 128], F32R, tag="tp", bufs=2,
                                      name=f"warm_{w}")
                        for i in range(4):
                            mm(wt[:, i, :], lhsT=id32_sb, rhs=id32_sb,
                               is_transpose=True, start=(i == 0), stop=(i == 3))
                    # xE natural [kr, d]: 2 krt x 2 dh chunks of [128, 512]
                    xE_ps = psE.tile([128, 2, 2, NSLAB], F32)  # 4 banks
                    for g in range(NG):
                        xt4 = frA.tile([128, 4, DIM], F16, tag="xn", bufs=3,
                                       name=f"xt_{g}")
                        nc.sync.dma_start(out=xt4, in_=xn_d[g])
                        et4 = frA.tile([128, 4, KR], F16, tag="et", bufs=3,
                                       name=f"et_{g}")
                        nc.sync.dma_start(out=et4, in_=Ed[g])
                        if g == 5:
                            nc.sync.dma_start(out=wk_sb, in_=Wk[:, :, :])
                            nc.sync.dma_start(out=wv_sb, in_=Wv[:, :, :])
                        elif g == 7:
                            nc.sync.dma_start(out=wo_sb, in_=Wo[:, :, :])
                        for i in range(4):
                            for krt in range(2):
                                for dh in range(2):
                                    mm(xE_ps[:, krt, dh, :],
                                       lhsT=et4[:, i, krt * 128:(krt + 1) * 128],
                                       rhs=xt4[:, i, dh * NSLAB:(dh + 1) * NSLAB],
                                       start=(g == 0 and i == 0),
                                       stop=(g == NG - 1 and i == 3))
                        if g in q_front_sched:
                            qT_front(*q_front_sched[g])
                    xE_sb = frA.tile([128, 2, DIM], F32R)
                    for krt in range(2):
                        rr_copy(krt, xE_sb[:, krt, :], xE_ps[:, krt, :, :])
                    # transpose xE [kr, d] -> xET [d, kr]: 16 blocks via PE
                    for grp in range(4):
                        tp = psQ.tile([128, 4, 128], F32R, tag="tp", bufs=2,
                                      name=f"tpx_{grp}")
                        for i in range(4):
                            blk = grp * 4 + i
                            dsub, krt = blk // 2, blk % 2
                            mm(tp[:, i, :],
                               lhsT=xE_sb[:, krt, dsub * 128:(dsub + 1) * 128],
                               rhs=id32_sb, is_transpose=True,
                               start=(i == 0), stop=(i == 3))
                        for i in range(4):
                            blk = grp * 4 + i
                            dsub, krt = blk // 2, blk % 2
                            rr_copy(blk, xET_sb[:, dsub, krt * 128:(krt + 1) * 128],
                                    tp[:, i, :])

                # kpT / vp from xET (reuses the 4 banks freed by psE)
                with tc.tile_pool(name="psKV", bufs=1, space="PSUM") as psKV:
                    kpT_ps = psKV.tile([128, 4, KR], F32)
                    for dc in range(DC):
                        for ct in range(4):
                            mm(kpT_ps[:, ct, :],
                               lhsT=wk_sb[:, dc, ct * 128:(ct + 1) * 128],
                               rhs=xET_sb[:, dc, :],
                               start=(dc == 0 and ct % 2 == 0),
                               stop=(dc == DC - 1))
                    nc.vector.tensor_copy(kpT_sb, kpT_ps)
                    vp_ps = psKV.tile([128, 2, CG], F32)
                    for dc in range(DC):
                        for krt in range(2):
                            mm(vp_ps[:, krt, :],
                               lhsT=xET_sb[:, dc, krt * 128:(krt + 1) * 128],
                               rhs=wv_sb[:, dc, :],
                               start=(dc == 0), stop=(dc == DC - 1))
                    nc.vector.tensor_copy(vp_sb, vp_ps)

            if dbg:
                nc.sync.dma_start(out=dbg_xET[:, :, :], in_=xET_sb)
                nc.sync.dma_start(out=dbg_kpT[:, :, :], in_=kpT_sb)
                nc.sync.dma_start(out=dbg_vp[:, :, :], in_=vp_sb)
                nc.sync.dma_start(out=dbg_qt[:, :, :, :], in_=qt[:, :, 0:2, :])

            # ---------------- HEADS epoch (software-pipelined) --------------
            with tc.tile_pool(name="hp", bufs=1) as hp, \
                 tc.tile_pool(name="psH", bufs=1, space="PSUM") as psH:
                outU = hp.tile([128, 4, 2, NSLAB], F16)  # 2-slab ring

                NSTEP = SLABS * 8

                def stage_scores(t):
                    # emit the head pair (t, t+1) interleaved: rows 0-63 and
                    # 64-127 are disjoint PE row groups -> concurrent matmuls
                    s, ct_h = t // 8, (t % 8) // 2
                    kA = kpT_sb[0:64, ct_h, :]
                    kB = kpT_sb[64:128, ct_h, :]
                    state[t]["natp"] = []
                    state[t + 1]["natp"] = []
                    for hf in range(2):
                        nA = psH.tile([128, 2, 1, 1, KR], F32, tag="nat",
                                      bufs=3, name=f"nat_{t}_{hf}")
                        nB = psH.tile([128, 2, 1, 1, KR], F32, tag="nat",
                                      bufs=3, name=f"nat_{t + 1}_{hf}")
                        for i in range(2):
                            ns = hf * 2 + i
                            qA = qt[0:64, ct_h, s, ns * 128:(ns + 1) * 128]
                            qB = qt[64:128, ct_h, s, ns * 128:(ns + 1) * 128]
                            mm(nA[:, i, 0, 0, :], lhsT=qA, rhs=kA,
                               start=(i == 0), stop=(i == 1),
                               skip_group_check=True)
                            mm(nB[:, i, 0, 0, :], lhsT=qB, rhs=kB,
                               start=(i == 0), stop=(i == 1),
                               skip_group_check=True)
                        state[t]["natp"].append(nA)
                        state[t + 1]["natp"].append(nB)

                def stage_soft(t):
                    # pool_max/pool_avg instead of tensor_reduce; bias is
                    # -max - ln(256) so exp emits U/256 and pool_avg's mean
                    # equals sum/256: the 256 cancels in U/denom exactly.
                    mrows = hp.tile([128, 4], F32, tag="mrows", bufs=3,
                                    name=f"mrows_{t}")
                    U_nat = hp.tile([128, 4, 1, 1, KR], F16, tag="unat",
                                    bufs=2, name=f"unat_{t}")
                    denom = hp.tile([128, 4], F32, tag="denom", bufs=2,
                                    name=f"denom_{t}")
                    for hf in range(2):
                        natp = state[t]["natp"][hf]
                        nc.vector.reduce_max(mrows[:, 2 * hf:2 * hf + 2],
                                             natp[:, :, 0, 0, :],
                                             axis=AXX, negate=True)
                        for i in range(2):
                            ns = hf * 2 + i
                            nc.scalar.activation(U_nat[:, ns, 0, 0, :],
                                                 natp[:, i, 0, 0, :],
                                                 EXP, bias=mrows[:, ns:ns + 1],
                                                 scale=1.0)
                    nc.vector.reduce_sum(denom, U_nat[:, :, 0, 0, :], axis=AXX)
                    srecip = hp.tile([128, 4], F32, tag="srecip", bufs=2,
                                     name=f"srecip_{t}")
                    nc.vector.reciprocal(srecip, denom)
                    U_norm = hp.tile([128, 4, KR], F16, tag="unorm", bufs=3,
                                     name=f"unorm_{t}")
                    for ns in range(4):
                        nc.vector.tensor_scalar_mul(U_norm[:, ns, :],
                                                    U_nat[:, ns, 0, 0, :],
                                                    srecip[:, ns:ns + 1])
                    return U_norm

                def stage_T(t):
                    U_norm = state[t]["U_norm"]
                    UT_ps = psH.tile([128, 2, NSLAB], F16, tag="utps", bufs=1,
                                     name=f"utps_{t}")
                    for ns in range(4):
                        for kb in range(2):
                            mm(UT_ps[:, kb, ns * 128:(ns + 1) * 128],
                               lhsT=U_norm[:, ns, kb * 128:(kb + 1) * 128],
                               rhs=id16_sb, is_transpose=True,
                               start=(ns == 0 and kb == 0),
                               stop=(ns == 3 and kb == 1))
                    UT_sb = hp.tile([128, 2, NSLAB], F16, tag="utsb", bufs=2,
                                    name=f"utsb_{t}")
                    if t % 2 == 0:
                        nc.vector.tensor_copy(UT_sb, UT_ps)
                    else:
                        nc.scalar.copy(UT_sb, UT_ps)
                    return UT_sb

                def stage_av_pair(t):
                    # t is the odd (second) head of the pair; emit both heads
                    # interleaved: out col groups 0-63 / 64-127 run
                    # concurrently on the PE; per-element has_written
                    # semantics make the interleaved start flags safe
                    s, h = t // 8, t % 8
                    ct_h = h // 2
                    hA, hB = h - 1, h
                    utA = state[t - 1]["UT_sb"]
                    utB = state[t]["UT_sb"]
                    av_ps = psH.tile([128, NSLAB], F32, tag="av", bufs=2,
                                     name=f"av_{t}")
                    for krt in range(2):
                        mm(av_ps[0:64, :],
                           lhsT=vp_sb[:, krt, hA * 64:(hA + 1) * 64],
                           rhs=utA[:, krt, :],
                           start=(krt == 0), stop=(krt == 1),
                           skip_group_check=True)
                        mm(av_ps[64:128, :],
                           lhsT=vp_sb[:, krt, hB * 64:(hB + 1) * 64],
                           rhs=utB[:, krt, :],
                           start=(krt == 0), stop=(krt == 1),
                           skip_group_check=True)
                    nc.scalar.copy(outU[:, ct_h, s % 2, :], av_ps)

                def stage_C(s, jc, ot):
                    f_ps = psH.tile([128, NSLAB], F32, tag="fps", bufs=2,
                                    name=f"fps_{s}_{jc}")
                    for ct in range(4):
                        mm(f_ps, lhsT=wo_sb[:, ct, jc * 128:(jc + 1) * 128],
                           rhs=outU[:, ct, s % 2, :],
                           start=(ct == 0), stop=(ct == 3))
                    nc.scalar.copy(ot[:, jc, :], f_ps)

                def stage_qT(s, ct, xs):
                    q_ps = psH.tile([128, NSLAB], F32, tag="av", bufs=2,
                                    name=f"qps_{s}_{ct}")
                    for dc in range(DC):
                        mm(q_ps, lhsT=wq_sb[:, dc, ct * 128:(ct + 1) * 128],
                           rhs=xs[:, dc, :], start=(dc == 0), stop=(dc == DC - 1))
                    nc.scalar.copy(qt[:, ct, s, :], q_ps)

                # C(s-1) chunk schedule: 8 jc chunks over steps h=3..7 of slab s
                c_sched = {3: [0, 1], 4: [2, 3], 5: [4], 6: [5], 7: [6, 7]}
                # qT(s+2) schedule: 4 ct chunks over steps h=0..3 of slab s
                q_sched = {0: [0], 1: [1], 2: [2], 3: [3]}

                xs_ring = {}
                ot_ring = {}

                def get_ot(u):
                    if u not in ot_ring:
                        ot_ring[u] = hp.tile([128, 8, NSLAB], F16, tag="ot",
                                             bufs=2, name=f"ot_{u}")
                    return ot_ring[u]

                state = {t: {} for t in range(NSTEP)}
                for t in range(NSTEP):
                    s, h = t // 8, t % 8
                    if h == 0 and s + 2 < SLABS:
                        xs = hp.tile([128, DC, NSLAB], F16, tag="xs2",
                                     bufs=2, name=f"xs2_{s + 2}")
                        nc.sync.dma_start(out=xs, in_=xT_d[s + 2])
                        xs_ring[s + 2] = xs
                    if t % 2 == 0:
                        stage_scores(t)
                    state[t]["U_norm"] = stage_soft(t)
                    if t - 2 >= 0:
                        state[t - 2]["UT_sb"] = stage_T(t - 2)
                    if t - 3 >= 0 and (t - 3) % 2 == 1:
                        stage_av_pair(t - 3)
                    if s == 0:
                        for ct in q_sched.get(h, []):
                            stage_qT(1, ct, xs_f[1])
                    if s + 2 < SLABS:
                        for ct in q_sched.get(h, []):
                            stage_qT(s + 2, ct, xs_ring[s + 2])
                    if s >= 1:
                        for jc in c_sched.get(h, []):
                            stage_C(s - 1, jc, get_ot(s - 1))
                        if h == 7:
                            nc.sync.dma_start(out=out_d[s - 1],
                                              in_=ot_ring[s - 1])
                # epilogue
                for t in (NSTEP - 2, NSTEP - 1):
                    state[t]["UT_sb"] = stage_T(t)
                stage_av_pair(NSTEP - 3)
                stage_av_pair(NSTEP - 1)
                for jc in range(8):
                    stage_C(SLABS - 1, jc, get_ot(SLABS - 1))
                nc.sync.dma_start(out=out_d[SLABS - 1], in_=ot_ring[SLABS - 1])

    nc.compile()
    return nc


def kernel(x, W_qkv, E, W_out, b_out):
    x = np.ascontiguousarray(np.asarray(x, dtype=np.float32))
    W_qkv = np.asarray(W_qkv, dtype=np.float32)
    E_np = np.asarray(E, dtype=np.float32)
    W_out = np.asarray(W_out, dtype=np.float32)
    b_out = np.asarray(b_out, dtype=np.float32)

    if "nc" not in _cache:
        _cache["nc"] = build_program()
    nc = _cache["nc"]

    E_t = np.ascontiguousarray(
        E_np.astype(np.float16).reshape(NG, 4, 128, KR).transpose(0, 2, 1, 3))
    id32 = np.eye(128, dtype=np.float32)
    id16 = np.eye(128, dtype=np.float16)
    in_maps = []
    for core in range(8):
        b, g = core // 2, core % 2
        cols = slice(g * CG, (g + 1) * CG)
        xb16 = x[b].astype(np.float16)
        xn_t = np.ascontiguousarray(
            xb16.reshape(NG, 4, 128, DIM).transpose(0, 2, 1, 3))
        xT_t = np.ascontiguousarray(
            xb16.T.reshape(DC, 128, SLABS, NSLAB).transpose(2, 1, 0, 3))
        Wq_t = np.ascontiguousarray(
            (W_qkv[:, 0 * DIM:1 * DIM][:, cols] * SCALE).astype(
                np.float16).reshape(DC, 128, CG).transpose(1, 0, 2))
        Wk_t = np.ascontiguousarray(
            W_qkv[:, 1 * DIM:2 * DIM][:, cols].reshape(
                DC, 128, CG).transpose(1, 0, 2))
        Wv_t = np.ascontiguousarray(
            W_qkv[:, 2 * DIM:3 * DIM][:, cols].reshape(
                DC, 128, CG).transpose(1, 0, 2))
        Wo_t = np.ascontiguousarray(
            W_out[g * CG:(g + 1) * CG, :].astype(np.float16).reshape(
                4, 128, DIM).transpose(1, 0, 2))
        in_maps.append({
            "xn": xn_t, "xT": xT_t, "E": E_t, "Wq": Wq_t, "Wk": Wk_t,
            "Wv": Wv_t, "Wo": Wo_t, "id32": id32, "id16": id16,
        })

    trace = bool(int(os.environ.get("KERNEL_TRACE", "0")))
    res = run_bass_kernel_spmd(nc, in_maps, core_ids=list(range(8)), trace=trace)
    _cache["last_results"] = res

    # partials come back as [slab, 128, jc, n] fp16; sum per batch in f32
    accT = np.zeros((4, DIM, SEQ), dtype=np.float32)
    for core in range(8):
        o = res.results[core]["out"].astype(np.float32)  # [S, 128, 8, N]
        accT[core // 2] += o.transpose(2, 1, 0, 3).reshape(DIM, SEQ)
    out = np.ascontiguousarray(accT.transpose(0, 2, 1))
    out += b_out[None, None, :]
    return out
